# revision 26
# baseline (speedup 1.0000x reference)
"""Trainium2 Bass kernel for the 2-layer Mamba-style model (nn_CAR_61143154425900).

Sharding: 8 cores = 4 batches x 2 d_inner-halves. Each core computes the full
token-parallel pipeline for its batch (duplicated within the pair for the
d_model-wide parts), runs the selective scan on its 256 d_inner channels via
the DVE tensor_tensor_scan instruction (partition = (d,s) rows, time in the
free dim), and contributes a partial output projection that is pair-AllReduced
on device. Host assembles logits + l2 loss.
"""
import functools
import os
import numpy as np

B = 4
L = 1024
NT = 8            # time tiles of 128
XD = 768
DM = 256          # d_model
DIN = 512         # d_inner (full)
OWN = 256         # d_inner channels per core
DS = 16           # d_state
DTR = 16          # dt_rank
NK = 32           # scan tiles per core (OWN*DS / 128)
NL = 2
EPS = 1e-5
L2_LAMBDA = 0.01
DCONV = 4


def _manifest_r0():
    return [(f"inw{k}", DM) for k in range(6)]


def _manifest_r():
    ents = []
    for l in range(NL):
        for k in range(2):
            ents.append((f"ipw{l}{k}", 768))
    for l in range(NL):
        for c in range(4):
            for j in range(DCONV):
                ents.append((f"cwd{l}{c}{j}", 128))
    for l in range(NL):
        for c in range(4):
            ents.append((f"xpw{l}{c}", 48))
    for l in range(NL):
        ents.append((f"dtw{l}", OWN))
    for m in range(16):
        ents.append((f"s8_{m}", 128))
    for m in range(16):
        ents.append((f"sR_{m}", 128))
    ents.append(("sB", 128))
    for l in range(NL):
        for d in range(2):
            ents.append((f"opw{l}{d}", DM))
    for k in range(2):
        ents.append((f"ow{k}", DM))
    for k in range(2):
        ents.append((f"cls{k}", 64))
    ents.append(("one1", 1))
    return ents


def _manifest_f():
    ents = [("inb", DM), ("l1g", DM), ("l1b", DM)]
    for l in range(NL):
        ents.append((f"ng{l}", DM))
        ents.append((f"nb{l}", DM))
    ents.append(("ob", DM))
    ents.append(("acols", NL * NK))
    for l in range(NL):
        for c in range(6):
            ents.append((f"xzb{l}{c}", 1))
    for l in range(NL):
        for c in range(4):
            ents.append((f"cb{l}{c}", 1))
    for l in range(NL):
        for d in range(2):
            ents.append((f"dtb{l}{d}", 1))
    for l in range(NL):
        for d in range(2):
            ents.append((f"D{l}{d}", 1))
    return ents


RW0 = sum(w for _, w in _manifest_r0())
RW = sum(w for _, w in _manifest_r())
FW = sum(w for _, w in _manifest_f())


def _build():
    from contextlib import ExitStack

    import concourse.bass as bass
    import concourse.mybir as mybir
    import concourse.tile as tile
    from concourse import bacc
    from concourse.masks import make_identity

    F32 = mybir.dt.float32
    F32R = mybir.dt.float32r
    AF = mybir.ActivationFunctionType
    OP = mybir.AluOpType
    _simsafe = os.environ.get("BASS_SIM_SAFE") == "1"
    AF_GELU = AF.Tanh if _simsafe else AF.Gelu
    AF_SILU = AF.Sigmoid if _simsafe else AF.Silu

    nc = bacc.Bacc("TRN2", target_bir_lowering=False, debug=False, num_devices=8)

    xT = nc.dram_tensor("xT", [128, 6 * L], F32R, kind="ExternalInput").ap()
    wr0 = nc.dram_tensor("wr0", [128, RW0], F32R, kind="ExternalInput").ap()
    wr = nc.dram_tensor("wr", [128, RW], F32R, kind="ExternalInput").ap()
    wf = nc.dram_tensor("wf", [128, FW], F32, kind="ExternalInput").ap()

    _dbg = os.environ.get("BASS_DEBUG_OUT") == "1"
    dbg = {}
    if _dbg:
        names = ["h0"]
        for l in range(NL):
            names += [f"l{l}_{n}" for n in
                      ["hlnT0", "u0", "sz0", "xd", "delta0", "du0", "Brep", "Crep",
                       "a0", "b0", "s0", "yc0", "ypar", "h"]]
        for n in names:
            wid = 2048 if n in ("h0",) or n.endswith("ypar") or n.endswith("_h") else L
            dbg[n] = nc.dram_tensor(f"dbg_{n}", [128, wid], F32,
                                    kind="ExternalOutput").ap()
    logitsT = nc.dram_tensor("logitsT", [64, L], F32, kind="ExternalOutput").ap()
    hnorm = nc.dram_tensor("hnorm", [128, NT], F32, kind="ExternalOutput").ap()

    ccin = nc.dram_tensor("ccin", [128, NT * DM], F32).ap()
    ccout = nc.dram_tensor("ccout", [128, NT * DM], F32).ap()

    with tile.TileContext(nc) as tc, ExitStack() as ctx:
        consts = ctx.enter_context(tc.tile_pool(name="consts", bufs=1))
        persist = ctx.enter_context(tc.tile_pool(name="persist", bufs=1))

        # weight blobs; wr0 (prologue weights) loads first
        Rt0 = consts.tile([128, RW0], F32R)
        nc.sync.dma_start(Rt0[:], wr0)
        Ft = consts.tile([128, FW], F32)
        nc.sync.dma_start(Ft[:], wf)
        Rt = consts.tile([128, RW], F32R)
        nc.sync.dma_start(Rt[:], wr)

        W = {}
        off = 0
        for nm, w in _manifest_r0():
            W[nm] = Rt0[:, off:off + w]
            off += w
        off = 0
        for nm, w in _manifest_r():
            W[nm] = Rt[:, off:off + w]
            off += w
        off = 0
        for nm, w in _manifest_f():
            W[nm] = Ft[:, off:off + w]
            off += w

        ident = consts.tile([128, 128], F32)
        make_identity(nc, ident[:])
        eps_t = consts.tile([128, 1], F32)
        nc.vector.memset(eps_t[:], EPS)

        h = persist.tile([128, NT * DM], F32)

        def hsl(tt):
            return h[:, tt * DM:(tt + 1) * DM]

        def xsl(k, tt):
            return xT_sb[:, k * L + tt * 128: k * L + (tt + 1) * 128]

        def ln_stats(pool, src_ap, mean8, var8, tt):
            """bn stats -> mean8[:, tt], var8[:, tt] (no ACT here)."""
            stats = pool.tile([128, 6], F32, tag="st", name="st")
            mv = pool.tile([128, 2], F32, tag="mv", name="mv")
            nc.vector.bn_stats(stats[:], src_ap)
            nc.vector.bn_aggr(mv[:], stats[:])
            nc.vector.tensor_copy(mean8[:, tt:tt + 1], mv[:, 0:1])
            nc.vector.tensor_copy(var8[:, tt:tt + 1], mv[:, 1:2])

        def ln_finish(pool, var8, rstd8):
            """rstd8 = exp(-0.5*ln(var8+eps)) -- one Ln + one Exp."""
            lv = pool.tile([128, NT], F32, tag="lv8", name="lv8")
            nc.scalar.activation(lv[:], var8[:], AF.Ln, bias=eps_t[:])
            nc.scalar.activation(rstd8[:], lv[:], AF.Exp, scale=-0.5)

        def ln_apply(pool, src_ap, mean8, rstd8, tt, g_ap, b_ap, out_ap):
            t1 = pool.tile([128, DM], F32, tag="t1", name="t1")
            nc.vector.tensor_scalar(
                t1[:], src_ap, mean8[:, tt:tt + 1], rstd8[:, tt:tt + 1],
                OP.subtract, OP.mult)
            nc.vector.scalar_tensor_tensor(
                out_ap, t1[:], 1.0, b_ap, OP.bypass, OP.add) if False else None
            nc.vector.scalar_tensor_tensor(
                out_ap, t1[:], 1.0, b_ap, OP.mult, OP.add)

        # ---------------- prologue ----------------
        with (
            tc.tile_pool(name="pro_ps", bufs=4, space="PSUM") as pro_ps,
            tc.tile_pool(name="pro_sb", bufs=2) as pro_sb,
            tc.tile_pool(name="pro_c", bufs=1) as pro_c,
        ):
            xT_sb = pro_c.tile([128, 6 * L], F32R)
            nc.sync.dma_start(xT_sb[:], xT)
            for tt in range(NT):
                ps = pro_ps.tile([128, DM], F32, tag="ps", name="ps")
                for k in range(6):
                    nc.tensor.matmul(
                        ps[:], xsl(k, tt), W[f"inw{k}"],
                        start=(k == 0), stop=(k == 5),
                    )
                nc.vector.tensor_tensor(hsl(tt), ps[:], W["inb"], OP.add)
            mean8 = pro_c.tile([128, NT], F32)
            var8 = pro_c.tile([128, NT], F32)
            rstd8 = pro_c.tile([128, NT], F32)
            for tt in range(NT):
                ln_stats(pro_sb, hsl(tt), mean8, var8, tt)
            ln_finish(pro_c, var8, rstd8)
            for tt in range(NT):
                t1 = pro_sb.tile([128, DM], F32, tag="t1", name="t1")
                nc.vector.tensor_scalar(
                    t1[:], hsl(tt), mean8[:, tt:tt + 1], rstd8[:, tt:tt + 1],
                    OP.subtract, OP.mult)
                t2 = pro_sb.tile([128, DM], F32, tag="t2", name="t2")
                nc.vector.scalar_tensor_tensor(
                    t2[:], t1[:], 1.0, W["l1g"], OP.mult, OP.mult)
                t3 = pro_sb.tile([128, DM], F32, tag="t3", name="t3")
                nc.vector.tensor_tensor(t3[:], t2[:], W["l1b"], OP.add)
                nc.scalar.activation(hsl(tt), t3[:], AF_GELU)

        if _dbg:
            nc.sync.dma_start(dbg["h0"], h[:])

        # ---------------- layers ----------------
        for l in range(NL):
            with (
                tc.tile_pool(name=f"l{l}_sb", bufs=1) as lsb,
                tc.tile_pool(name=f"l{l}_tmp", bufs=2) as ltmp,
            ):
                sz = [lsb.tile([128, L], F32, tag=f"sz{c}", name=f"sz{c}")
                      for c in range(2)]
                uT_act = [lsb.tile([128, L], F32R, tag=f"ua{c}", name=f"ua{c}")
                          for c in range(4)]
                deltaT = [lsb.tile([128, L], F32R, tag=f"de{d}", name=f"de{d}")
                          for d in range(2)]
                duT = [lsb.tile([128, L], F32R, tag=f"du{d}", name=f"du{d}")
                       for d in range(2)]
                B_rep = lsb.tile([128, L], F32)
                C_rep = lsb.tile([128, L], F32)
                y_f = [lsb.tile([128, L], F32R, tag=f"yf{d}", name=f"yf{d}")
                       for d in range(2)]

                with (
                    tc.tile_pool(name=f"l{l}_pre", bufs=1) as pre,
                    tc.tile_pool(name=f"l{l}_ps", bufs=4, space="PSUM") as lps,
                ):
                    # LN stats (A table), then apply + transpose
                    mean8 = pre.tile([128, NT], F32, name="mean8")
                    var8 = pre.tile([128, NT], F32, name="var8")
                    rstd8 = pre.tile([128, NT], F32, name="rstd8")
                    for tt in range(NT):
                        ln_stats(ltmp, hsl(tt), mean8, var8, tt)
                    ln_finish(pre, var8, rstd8)
                    hlnT = [pre.tile([128, L], F32R, tag=f"hlnT{k}", name=f"hlnT{k}")
                            for k in range(2)]
                    for tt in range(NT):
                        hln = ltmp.tile([128, DM], F32, tag="hln", name="hln")
                        nc.vector.tensor_scalar(
                            hln[:], hsl(tt), mean8[:, tt:tt + 1], rstd8[:, tt:tt + 1],
                            OP.subtract, OP.mult)
                        for k in range(2):
                            pt = lps.tile([128, 512], F32, tag="mm", name="pt")
                            nc.tensor.transpose(
                                pt[:, 0:128], hln[:, k * 128:(k + 1) * 128], ident[:])
                            nc.scalar.copy(
                                hlnT[k][:, tt * 128:(tt + 1) * 128], pt[:, 0:128])

                    # xz^T = ipw_eff @ hln^T : 6 ch-tiles (u own, u other, z own)
                    u_pad = [pre.tile([128, 3 + L], F32R, tag=f"up{c}", name=f"up{c}")
                             for c in range(4)]
                    for c in range(4):
                        nc.vector.memset(u_pad[c][:, 0:3].bitcast(F32), 0.0)
                    for c in range(6):
                        for nn in range(2):
                            ps = lps.tile([128, 512], F32, tag="mm", name="ps")
                            for k in range(2):
                                nc.tensor.matmul(
                                    ps[:],
                                    W[f"ipw{l}{k}"][:, c * 128:(c + 1) * 128],
                                    hlnT[k][:, nn * 512:(nn + 1) * 512],
                                    start=(k == 0), stop=(k == 1),
                                )
                            if c < 4:
                                nc.scalar.activation(
                                    u_pad[c][:, 3 + nn * 512:3 + (nn + 1) * 512],
                                    ps[:], AF.Identity, bias=W[f"xzb{l}{c}"])
                            else:
                                nc.scalar.activation(
                                    sz[c - 4][:, nn * 512:(nn + 1) * 512],
                                    ps[:], AF.Identity, bias=W[f"xzb{l}{c}"])

                    # causal depthwise conv on PE (diag matmuls) + silu -> uT_act
                    for c in range(4):
                        for nn in range(2):
                            psu = lps.tile([128, 512], F32, tag="mm", name="psu")
                            for j in range(DCONV):
                                nc.tensor.matmul(
                                    psu[:], W[f"cwd{l}{c}{j}"],
                                    u_pad[c][:, j + nn * 512:j + nn * 512 + 512],
                                    start=(j == 0), stop=(j == DCONV - 1))
                            nc.scalar.activation(
                                uT_act[c][:, nn * 512:(nn + 1) * 512], psu[:],
                                AF_SILU, bias=W[f"cb{l}{c}"])

                    # xdbc^T: packed tile rows 0:16 dt, 32:48 B, 64:80 C
                    xd = pre.tile([128, L], F32R, name="xd")
                    if _dbg:
                        nc.vector.memset(xd[:].bitcast(F32), 0.0)
                    for nn in range(2):
                        sl = slice(nn * 512, (nn + 1) * 512)
                        for j, xdoff in enumerate((0, 32, 64)):
                            ps = lps.tile([128, 512], F32, tag="mm", name="ps48")
                            for c in range(4):
                                nc.tensor.matmul(
                                    ps[0:16, :], W[f"xpw{l}{c}"][:, j * 16:(j + 1) * 16],
                                    uT_act[c][:, sl],
                                    start=(c == 0), stop=(c == 3),
                                )
                            nc.scalar.copy(xd[xdoff:xdoff + 16, sl], ps[0:16, :])

                    # delta^T = softplus(dtw @ dt^T + dtb): Exp pass then Ln pass
                    esbs = []
                    for d in range(2):
                        esb = pre.tile([128, L], F32, tag=f"sp{d}", name=f"sp{d}")
                        esbs.append(esb)
                        for nn in range(2):
                            ps = lps.tile([128, 512], F32, tag="mm", name="psd")
                            nc.tensor.matmul(
                                ps[:], W[f"dtw{l}"][0:DTR, d * 128:(d + 1) * 128],
                                xd[0:DTR, nn * 512:(nn + 1) * 512],
                                start=True, stop=True,
                            )
                            nc.scalar.activation(
                                esb[:, nn * 512:(nn + 1) * 512], ps[:], AF.Exp,
                                bias=W[f"dtb{l}{d}"])
                    for d in range(2):
                        nc.scalar.activation(deltaT[d][:], esbs[d][:], AF.Ln, bias=1.0)
                        nc.vector.tensor_tensor(
                            duT[d][:], deltaT[d][:], uT_act[d][:], OP.mult)

                    # B_rep / C_rep
                    for nn in range(2):
                        sl = slice(nn * 512, (nn + 1) * 512)
                        psb = lps.tile([128, 512], F32, tag="mm", name="psb")
                        nc.tensor.matmul(psb[:], W["sB"][32:32 + DS, :],
                                         xd[32:32 + DS, sl], start=True, stop=True)
                        nc.scalar.copy(B_rep[:, sl], psb[:])
                        psc = lps.tile([128, 512], F32, tag="mm", name="psc")
                        nc.tensor.matmul(psc[:], W["sB"][64:64 + DS, :],
                                         xd[64:64 + DS, sl], start=True, stop=True)
                        nc.scalar.copy(C_rep[:, sl], psc[:])

                    if _dbg:
                        nc.gpsimd.dma_start(dbg[f"l{l}_hlnT0"], hlnT[0][:])
                        nc.gpsimd.dma_start(dbg[f"l{l}_u0"], uT_act[0][:])
                        nc.sync.dma_start(dbg[f"l{l}_sz0"], sz[0][:])
                        nc.gpsimd.dma_start(dbg[f"l{l}_xd"], xd[:])
                        nc.gpsimd.dma_start(dbg[f"l{l}_delta0"], deltaT[0][:])
                        nc.gpsimd.dma_start(dbg[f"l{l}_du0"], duT[0][:])
                        nc.sync.dma_start(dbg[f"l{l}_Brep"], B_rep[:])
                        nc.sync.dma_start(dbg[f"l{l}_Crep"], C_rep[:])

                # ---- scan ----  [A table: Exp]
                with (
                    tc.tile_pool(name=f"l{l}_scanps", bufs=3, space="PSUM") as sps,
                    tc.tile_pool(name=f"l{l}_yps", bufs=1, space="PSUM") as yps,
                    tc.tile_pool(name=f"l{l}_scansb", bufs=2) as ssb,
                ):
                    for dt_i in range(2):
                        ps_y = yps.tile([128, L], F32, tag="psy", name="psy")
                        for m in range(16):
                            k = dt_i * 16 + m
                            a_sb = ssb.tile([128, L], F32, tag="a", name="a")
                            b_sb = ssb.tile([128, L], F32, tag="b", name="b")
                            pa = sps.tile([128, L], F32, tag="pp", name="pa")
                            pd = sps.tile([128, L], F32, tag="pp", name="pd")
                            for nn in range(2):
                                sl = slice(nn * 512, (nn + 1) * 512)
                                nc.tensor.matmul(
                                    pa[:, sl], W[f"s8_{m}"], deltaT[dt_i][:, sl],
                                    start=True, stop=True)
                                nc.tensor.matmul(
                                    pd[:, sl], W[f"s8_{m}"], duT[dt_i][:, sl],
                                    start=True, stop=True)
                            nc.scalar.activation(
                                a_sb[:], pa[:], AF.Exp,
                                scale=W["acols"][:, l * NK + k:l * NK + k + 1])
                            nc.vector.tensor_tensor(b_sb[:], pd[:], B_rep[:], OP.mult)
                            s_sb = ssb.tile([128, L], F32, tag="s", name="s")
                            nc.vector.tensor_tensor_scan(
                                s_sb[:], a_sb[:], b_sb[:], 0.0, OP.mult, OP.add)
                            yc = ssb.tile([128, L], F32R, tag="yc", name="yc")
                            nc.gpsimd.tensor_tensor(yc[:], s_sb[:], C_rep[:], OP.mult)
                            if _dbg and k == 0:
                                nc.sync.dma_start(dbg[f"l{l}_a0"], a_sb[:])
                                nc.sync.dma_start(dbg[f"l{l}_b0"], b_sb[:])
                                nc.sync.dma_start(dbg[f"l{l}_s0"], s_sb[:])
                                nc.gpsimd.dma_start(dbg[f"l{l}_yc0"], yc[:])
                            for nn in range(2):
                                sl = slice(nn * 512, (nn + 1) * 512)
                                nc.tensor.matmul(
                                    ps_y[:, sl], W[f"sR_{m}"], yc[:, sl],
                                    start=(m == 0), stop=(m == 15))

                        # y = (u*D + y_scan) * silu(z) -> f32r
                        nc.scalar.activation(sz[dt_i][:], sz[dt_i][:], AF_SILU)
                        y1 = ltmp.tile([128, L], F32, tag="y1", name="y1")
                        nc.vector.scalar_tensor_tensor(
                            y1[:], uT_act[dt_i][:], W[f"D{l}{dt_i}"], ps_y[:],
                            OP.mult, OP.add)
                        nc.gpsimd.tensor_tensor(
                            y_f[dt_i][:], y1[:], sz[dt_i][:], OP.mult)

                # opw partial: (t, dm) accumulated over 2 d-tiles
                y_par = lsb.tile([128, NT * DM], F32)
                with tc.tile_pool(name=f"l{l}_ops", bufs=4, space="PSUM") as ops_:
                    for tt in range(NT):
                        ps = ops_.tile([128, 512], F32, tag="mm", name="pso")
                        for d in range(2):
                            nc.tensor.matmul(
                                ps[:, 0:DM], y_f[d][:, tt * 128:(tt + 1) * 128],
                                W[f"opw{l}{d}"], start=(d == 0), stop=(d == 1))
                        nc.scalar.copy(y_par[:, tt * DM:(tt + 1) * DM], ps[:, 0:DM])

                # AllReduce over the pair, then residual add
                nc.sync.dma_start(ccin, y_par[:])
                if os.environ.get("BASS_NO_CC") == "1":
                    nc.sync.dma_start(ccout, ccin)
                else:
                    nc.gpsimd.collective_compute(
                        "AllReduce", OP.add,
                        replica_groups=[[0, 1], [2, 3], [4, 5], [6, 7]],
                        ins=[ccin], outs=[ccout],
                    )
                ar = lsb.tile([128, NT * DM], F32)
                nc.sync.dma_start(ar[:], ccout)
                for tt in range(NT):
                    nc.gpsimd.tensor_tensor(
                        hsl(tt), hsl(tt), ar[:, tt * DM:(tt + 1) * DM], OP.add)
                if _dbg:
                    nc.sync.dma_start(dbg[f"l{l}_ypar"], y_par[:])
                    nc.sync.dma_start(dbg[f"l{l}_h"], h[:])

        # ---------------- epilogue ----------------
        with (
            tc.tile_pool(name="ep_sb", bufs=1) as esb,
            tc.tile_pool(name="ep_tmp", bufs=3) as etmp,
            tc.tile_pool(name="ep_ps", bufs=4, space="PSUM") as eps_,
        ):
            hT = [esb.tile([128, L], F32R, tag=f"hT{k}", name=f"hT{k}")
                  for k in range(2)]
            for tt in range(NT):
                for k in range(2):
                    pt = eps_.tile([128, 512], F32, tag="mm", name="pt")
                    nc.tensor.transpose(
                        pt[:, 0:128],
                        h[:, tt * DM + k * 128: tt * DM + (k + 1) * 128], ident[:])
                    nc.scalar.copy(hT[k][:, tt * 128:(tt + 1) * 128], pt[:, 0:128])

            # matmul + bias; transpose h2 then gelu directly into hgT
            hgT = [esb.tile([128, L], F32R, tag=f"hgT{k}", name=f"hgT{k}")
                   for k in range(2)]
            for tt in range(NT):
                ps = eps_.tile([128, 512], F32, tag="mm", name="pse")
                for k in range(2):
                    nc.tensor.matmul(
                        ps[:, 0:DM], hT[k][:, tt * 128:(tt + 1) * 128], W[f"ow{k}"],
                        start=(k == 0), stop=(k == 1))
                h2 = etmp.tile([128, DM], F32, tag="h2", name="h2")
                nc.vector.tensor_tensor(h2[:], ps[:, 0:DM], W["ob"], OP.add)
                for k in range(2):
                    pt = eps_.tile([128, 512], F32, tag="mm", name="pt2")
                    nc.tensor.transpose(
                        pt[:, 0:128], h2[:, k * 128:(k + 1) * 128], ident[:])
                    nc.scalar.activation(
                        hgT[k][:, tt * 128:(tt + 1) * 128], pt[:, 0:128], AF_GELU)
            # norms: hg2 = hgT^2 (DVE), ones-matmul over dm, sqrt via Ln/Exp
            hg2 = [etmp.tile([128, L], F32R, tag=f"hg2{k}", name=f"hg2{k}")
                   for k in range(2)]
            for k in range(2):
                nc.vector.tensor_tensor(hg2[k][:], hgT[k][:], hgT[k][:], OP.mult)
            nrm_row = esb.tile([1, L], F32)
            for nn in range(2):
                psn2 = eps_.tile([128, 512], F32, tag="mm", name="psn2")
                for k in range(2):
                    nc.tensor.matmul(
                        psn2[0:1, :], W["one1"],
                        hg2[k][:, nn * 512:(nn + 1) * 512],
                        start=(k == 0), stop=(k == 1))
                nc.scalar.copy(nrm_row[:, nn * 512:(nn + 1) * 512], psn2[0:1, :])
            lnv = etmp.tile([1, L], F32, tag="lnv", name="lnv")
            nc.scalar.activation(lnv[:], nrm_row[:], AF.Ln)
            nrm = etmp.tile([1, L], F32, tag="nrm", name="nrm")
            nc.scalar.activation(nrm[:], lnv[:], AF.Exp, scale=0.5)
            nc.sync.dma_start(hnorm.rearrange("p f -> (p f)")[None, :], nrm[:])

            lg = esb.tile([64, L], F32)
            for nn in range(2):
                ps = eps_.tile([128, 512], F32, tag="mm", name="psl")
                for k in range(2):
                    nc.tensor.matmul(
                        ps[0:64, :], W[f"cls{k}"], hgT[k][:, nn * 512:(nn + 1) * 512],
                        start=(k == 0), stop=(k == 1))
                nc.scalar.copy(lg[:, nn * 512:(nn + 1) * 512], ps[0:64, :])
            nc.sync.dma_start(logitsT, lg[:])

    nc.compile()
    return nc


@functools.lru_cache(maxsize=1)
def _built():
    return _build()


def _host_prep(inputs):
    """Build per-core in_maps from full inputs."""
    x = np.asarray(inputs["x"], np.float32)
    in_w = np.asarray(inputs["in_w"], np.float32)
    in_b = np.asarray(inputs["in_b"], np.float32)
    ln_g = np.asarray(inputs["ln_g"], np.float32)
    ln_b = np.asarray(inputs["ln_b"], np.float32)
    blk_ng = np.asarray(inputs["blk_ng"], np.float32)
    blk_nb = np.asarray(inputs["blk_nb"], np.float32)
    blk_ipw = np.asarray(inputs["blk_ipw"], np.float32)
    blk_cw = np.asarray(inputs["blk_cw"], np.float32)
    blk_cb = np.asarray(inputs["blk_cb"], np.float32)
    blk_xpw = np.asarray(inputs["blk_xpw"], np.float32)
    blk_dtw = np.asarray(inputs["blk_dtw"], np.float32)
    blk_dtb = np.asarray(inputs["blk_dtb"], np.float32)
    blk_Alog = np.asarray(inputs["blk_Alog"], np.float32)
    blk_D = np.asarray(inputs["blk_D"], np.float32)
    blk_opw = np.asarray(inputs["blk_opw"], np.float32)
    op_w = np.asarray(inputs["op_w"], np.float32)
    op_b = np.asarray(inputs["op_b"], np.float32)
    cls_w = np.asarray(inputs["cls_w"], np.float32)

    sel8 = np.zeros((16, 128, 128), np.float32)
    selR = np.zeros((16, 128, 128), np.float32)
    for m in range(16):
        for p in range(128):
            sel8[m, 8 * m + p // 16, p] = 1.0
            selR[m, p, 8 * m + p // 16] = 1.0
    selB = np.zeros((128, 128), np.float32)
    for p in range(128):
        selB[32 + p % 16, p] = 1.0
        selB[64 + p % 16, p] = 1.0

    in_maps = []
    for c in range(8):
        b, half = c // 2, c % 2
        own = np.arange(OWN * half, OWN * half + OWN)
        other = np.arange(OWN * (1 - half), OWN * (1 - half) + OWN)
        perm = np.concatenate([own, other])

        R = {}
        R0 = {}
        F = {}
        inwT = in_w.T  # (768, 256)
        for k in range(6):
            R0[f"inw{k}"] = inwT[k * 128:(k + 1) * 128]
        for l in range(NL):
            u_rows = blk_ipw[l][:DIN][perm]
            z_rows = blk_ipw[l][DIN:][own]
            ipw_eff = np.concatenate([u_rows, z_rows], axis=0)  # (768, 256)
            xzb = ipw_eff @ blk_nb[l]                            # (768,)
            ipw_eff = ipw_eff * blk_ng[l][None, :]
            ipwT = ipw_eff.T                                     # (256, 768)
            for cc in range(6):
                F[f"xzb{l}{cc}"] = xzb[cc * 128:(cc + 1) * 128][:, None]
            for k in range(2):
                R[f"ipw{l}{k}"] = ipwT[k * 128:(k + 1) * 128]
            cwp = blk_cw[l][perm, 0, :]
            for cc in range(4):
                for j in range(DCONV):
                    dg = np.zeros((128, 128), np.float32)
                    np.fill_diagonal(dg, cwp[cc * 128:(cc + 1) * 128, j])
                    R[f"cwd{l}{cc}{j}"] = dg
            xpwT = blk_xpw[l][:, perm].T                        # (512, 48)
            for cc in range(4):
                R[f"xpw{l}{cc}"] = xpwT[cc * 128:(cc + 1) * 128]
            dtw = np.zeros((128, OWN), np.float32)
            dtw[0:DTR] = blk_dtw[l][own].T
            R[f"dtw{l}"] = dtw
            opwT = blk_opw[l][:, own].T                         # (256, 256)
            for d in range(2):
                R[f"opw{l}{d}"] = opwT[d * 128:(d + 1) * 128]
        for m in range(16):
            R[f"s8_{m}"] = sel8[m]
            R[f"sR_{m}"] = selR[m]
        R["sB"] = selB
        owT = op_w.T
        for k in range(2):
            R[f"ow{k}"] = owT[k * 128:(k + 1) * 128]
        clsT = cls_w[64 * half:64 * half + 64].T                # (256, 64)
        for k in range(2):
            R[f"cls{k}"] = clsT[k * 128:(k + 1) * 128]
        R["one1"] = np.ones((128, 1), np.float32)

        def rep(v):
            return np.tile(v[None, :], (128, 1)).astype(np.float32)

        F["inb"] = rep(in_b)
        F["l1g"] = rep(ln_g)
        F["l1b"] = rep(ln_b)
        acols = np.zeros((128, NL * NK), np.float32)
        for l in range(NL):
            F[f"ng{l}"] = rep(blk_ng[l])
            F[f"nb{l}"] = rep(blk_nb[l])
            A = -np.exp(blk_Alog[l][own])
            acols[:, l * NK:(l + 1) * NK] = A.reshape(NK, 128).T
            cbp = blk_cb[l][perm]
            for cc in range(4):
                F[f"cb{l}{cc}"] = cbp[cc * 128:(cc + 1) * 128][:, None]
            for d in range(2):
                F[f"dtb{l}{d}"] = blk_dtb[l][own][d * 128:(d + 1) * 128][:, None]
                F[f"D{l}{d}"] = blk_D[l][own][d * 128:(d + 1) * 128][:, None]
        F["ob"] = rep(op_b)
        F["acols"] = acols

        wr0c = np.concatenate([R0[nm] for nm, _ in _manifest_r0()], axis=1)
        wr = np.concatenate([R[nm] for nm, _ in _manifest_r()], axis=1)
        wf = np.concatenate([F[nm] for nm, _ in _manifest_f()], axis=1)
        xTb = np.ascontiguousarray(
            x[b].T.reshape(6, 128, L).transpose(1, 0, 2).reshape(128, 6 * L))

        in_maps.append({
            "xT": xTb,
            "wr0": np.ascontiguousarray(wr0c),
            "wr": np.ascontiguousarray(wr),
            "wf": np.ascontiguousarray(wf),
        })
    return in_maps


def kernel(**inputs):
    from concourse.bass_utils import run_bass_kernel_spmd

    nc = _built()
    in_maps = _host_prep(inputs)
    res = run_bass_kernel_spmd(nc, in_maps, list(range(8)))

    cls_b = np.asarray(inputs["cls_b"], np.float32)
    logits = np.zeros((B, L, 128), np.float32)
    norms = np.zeros((B, L), np.float32)
    for c in range(8):
        b, half = c // 2, c % 2
        r = res.results[c]
        logits[b, :, 64 * half:64 * half + 64] = r["logitsT"].T
        if half == 0:
            norms[b] = r["hnorm"].reshape(L)
    logits += cls_b[None, None, :]
    l2 = np.float32(L2_LAMBDA * norms.mean())
    return logits, l2
